# revision 1
# baseline (speedup 1.0000x reference)
"""MoE routing kernel for Trainium2 (8 NeuronCores, expert-parallel).

Problem: nn_MoDE_52140902973544 (moe_routing).
  x[4,2048,1024], router (8 experts, top-2, capacity 1024), 7 real experts
  with FFN H=1024 -> I=4096 -> H=1024 (relu), expert 7 = identity (noop).

Strategy:
  * Host: router forward + top-2 + capacity-limited dispatch (pure index
    math, order-based -> float-robust), gather dispatched tokens per
    expert transposed to [H, cap].
  * Device (SPMD over 8 cores): core e computes
        outT_e = (relu(disp_e @ Wi_e) @ Wo_e).T            # [H, cap]
    Core 7 duplicates core 0 (7 real experts); its output is ignored.
  * Host: combine via pure gathers (no scatter) + gate weights + noop path.

Device kernel layout (bf16 operands, fp32 PSUM):
  * x and h stay SBUF-resident; Wi arrives in 5 pieces (a tiny first piece
    so the PE starts after ~0.3MB of DMA) through a 4-slot ring that the
    4 Wo pieces rotate into as GEMM1 consumes them -> weight DMA fully
    overlaps compute.
  * GEMM1 interleaves the two 512-token PSUM tiles per i-chunk; GEMM2
    interleaves 4 PSUM tiles; outbound DMA is per-piece so only ~1MB of
    the fp32 output write is a serial tail.
  * Measured (slope method, see test.py): ~293us/execution at the chip's
    sustained-power clock. The same NEFF measured 209us/iter (0.96
    cycles/column at the full 2.4GHz) in a fresh-clock session right
    after a core reset — i.e. the kernel streams the PE at ~100%
    efficiency and the 293/209 gap is chip power-state throttle, not
    code. fp8 DoubleRow runs 2x faster but its e4m3 quantization noise
    (~4e-2 end-to-end) fails the 2e-2 gate, and residual-compensation
    schemes cost more than bf16.
"""

import os
import sys

for _p in ("/opt/trn_rl_repo", "/opt/pypackages"):
    if _p not in sys.path:
        sys.path.append(_p)

import numpy as np

# ---- problem constants (hardcoded per contract) ----
B, S, H, I = 4, 2048, 1024, 4096
E = 8                 # experts incl. noop (last)
ER = E - 1            # real experts
TOP_K = 2
N_TOK = B * S         # 8192
CAP = 1024            # ceil(N_TOK / E * 1.0)
N_CORES = 8

P = 128               # partitions
KO = H // P           # 8   H chunks
IC = I // P           # 32  I chunks
NF = 512              # matmul free dim
NN = CAP // NF        # 2   cap tiles

# matmul operand dtype: "bf16" (fast, host cast) or "fp8" (float8e4 +
# DoubleRow perf mode, ~2x PE throughput vs bf16 but ~4e-2 rel err —
# fails the 2e-2 gate; kept for experiments only)
MM_DTYPE = os.environ.get("MOE_MM_DTYPE", "bf16")
# fp8 pre-scales (powers of 2: exponent-only, no mantissa change) lift the
# small expert weights out of the fp8e4 subnormal range; the host combine
# divides the device output by WI_SCALE*WO_SCALE.
WI_SCALE = 16.0
WO_SCALE = 64.0

_CACHE = {}


def _build_nc(mm_dtype: str, repeat: int = 1, loop_repeat: int | None = None,
              staggered: bool = False, ablate: str = "full",
              x_one: bool = False, ps2bank: bool = True,
              wide_mm: bool = False):
    """Build the single-core Bass program (SPMD across 8 cores).

    Layout: x [H,cap] and h [I,cap] stay SBUF-resident in bf16; Wi is
    loaded in 4 pieces which the Wo pieces rotate into (pool tag ring)
    as GEMM1 consumes them, so weight DMA fully overlaps compute and the
    SBUF footprint stays ~176 KB/partition.  GEMM2 accumulates the full
    I contraction in PSUM (4 banks live) and a single outbound DMA
    writes outT.
    """
    import concourse.bacc as bacc
    import concourse.mybir as mybir
    import concourse.tile as tile

    dt = mybir.dt
    assert mm_dtype in ("bf16", "fp8")
    fp8 = mm_dtype == "fp8"
    # fp8: operands in float8e4 (TRN e4m3, max +-240), matmuls in DoubleRow
    # perf mode (2 k-chunks per instruction, ~1.4x PE throughput). PSUM
    # accumulation stays fp32; host pre-scales wi by 16 and wo by 64 to
    # center the fp8 dynamic range, and the host combine divides the gate
    # weights by 1024 to compensate (relu commutes with positive scale).
    DT = dt.float8e4 if fp8 else dt.bfloat16
    KSTEP = 2 if fp8 else 1
    PERF = mybir.MatmulPerfMode.DoubleRow if fp8 else None

    # Bacc (not raw Bass): its compile() pipeline splits multi-semaphore
    # waits into event-semaphore chains (TRN2 allows 1 wait/instruction)
    # and moves matmul waits onto ldweights.
    nc = bacc.Bacc("TRN2")
    xT = nc.declare_dram_parameter("xT", [H, CAP], DT, isOutput=False)
    wi = nc.declare_dram_parameter("wi", [H, I], DT, isOutput=False)
    wo = nc.declare_dram_parameter("wo", [I, H], DT, isOutput=False)
    outT = nc.declare_dram_parameter("outT", [H, CAP], dt.float32, isOutput=True)
    NPIECE = 4
    IPP = IC // NPIECE        # 8 i-chunks per wi piece

    with tile.TileContext(nc) as tc:
        from contextlib import ExitStack

        with ExitStack() as ctx:
            xpool = ctx.enter_context(tc.tile_pool(name="x", bufs=1))
            wpool = ctx.enter_context(tc.tile_pool(name="w", bufs=NPIECE))
            hpool = ctx.enter_context(tc.tile_pool(name="h", bufs=1))
            opool = ctx.enter_context(tc.tile_pool(name="o", bufs=1))
            # one shared 8-bank PSUM ring: GEMM1 draws 4 banks per i-chunk
            # pair, GEMM2 draws 4 per group — consecutive groups land on
            # different bank halves, so the DVE copies of group g overlap
            # the PE matmuls of group g+1 instead of serializing on the
            # same banks. ps2bank: tiles span 2 banks (both n-halves), so
            # each relu/copy drains a full [128,1024] row in one DVE op.
            pspool = ctx.enter_context(
                tc.tile_pool(name="ps", bufs=(4 if ps2bank else 8),
                             space="PSUM"))

            # x split into the two 512-token halves: GEMM1's first chain
            # only waits on the first 1MB DMA, not the full 2MB
            # (x_one=True: single tile, A/B experiment for SBUF access
            # patterns — rhs then comes from one tile for all matmuls)
            if x_one:
                x_whole = xpool.tile([P, KO, CAP], DT, name="x")
                x_sbs = [x_whole[:, :, n * NF:(n + 1) * NF] for n in range(NN)]
            else:
                x_sbs = [xpool.tile([P, KO, NF], DT, name=f"x{n}")
                         for n in range(NN)]
            # h split into two half-tiles: GEMM2's k=0..15 matmuls wait only
            # on the first half's relus (done ~70us before GEMM1 ends), so
            # the GEMM1->GEMM2 transition has no whole-tile barrier
            ICH = IC // 2
            h_sbs = [hpool.tile([P, ICH, CAP], DT, name=f"h{j}")
                     for j in range(2)]
            out_sb = opool.tile([P, KO, CAP], dt.float32)

            wi_r = wi.rearrange("(ko p) i -> p ko i", p=P)     # [128, 8, 4096]
            wo_r = wo.rearrange("(ki p) h -> p ki h", p=P)     # [128, 32, 1024]

          # fmt: off
          # noqa
            # wi piece sizes in i-chunks: a tiny first piece so the PE can
            # start GEMM1 after a ~0.5MB DMA instead of 4MB (the tile dep
            # is whole-tile); the ring still has NPIECE slots, with wi4
            # reusing wi0's slot after its chunks are consumed. All counts
            # even so GEMM1 can pair i-chunks for 4-bank interleaving.
            WI_SPLIT = [2, IC // NPIECE - 2] + [IC // NPIECE] * (NPIECE - 1)
            WI_START = [sum(WI_SPLIT[:j]) for j in range(len(WI_SPLIT))]

            xT_r = xT.rearrange("(ko p) n -> p ko n", p=P)

            # ablate: "full" | "empty" (loop overhead only) | "nog2"
            # (GEMM1+relu only) | "nog2pre" (GEMM1 only, wi preloaded
            # OUTSIDE the loop — separates DMA interference from relu
            # interference) | "notail" (skip PSUM->SBUF copies + out
            # DMA) | "nodma" (skip input DMAs; compute on stale SBUF) —
            # timing-attribution experiments, not for production use.
            do_dma = ablate != "nodma"
            do_g2 = ablate not in ("nog2", "nog2pre")
            do_tail = ablate not in ("nog2", "nog2pre", "notail")
            preload = ablate == "nog2pre"

            wi_pre = None
            if preload:
                # wi fully SBUF-resident (64KB/partition; fits since wo and
                # the w-ring go unused in this variant)
                wprepool = ctx.enter_context(
                    tc.tile_pool(name="wpre", bufs=1))
                wi_pre = []
                for p_, (i0, cnt) in enumerate(zip(WI_START, WI_SPLIT)):
                    wt = wprepool.tile([P, KO, cnt * P], DT,
                                       tag=f"wpre{p_}", name=f"wipre{p_}")
                    nc.sync.dma_start(
                        wt[:], wi_r[:, :, i0 * P:(i0 + cnt) * P])
                    wi_pre.append(wt)

            def _emit_body():
              if ablate == "empty":
                  nc.any.memset(out_sb[:, 0, 0:NF], 0.0)
                  return
              if do_dma:
                  if x_one:
                      nc.sync.dma_start(x_whole[:], xT_r[:])
                  else:
                      for n in range(NN):
                          nc.sync.dma_start(
                              x_sbs[n][:], xT_r[:, :, n * NF:(n + 1) * NF])
              if preload:
                  wi_pieces = wi_pre
              else:
                  wi_pieces = []
                  for p_, (i0, cnt) in enumerate(zip(WI_START, WI_SPLIT)):
                      wt = wpool.tile([P, KO, cnt * P], DT, tag="w",
                                      name=f"wi{p_}")
                      if do_dma:
                          nc.sync.dma_start(
                              wt[:], wi_r[:, :, i0 * P:(i0 + cnt) * P])
                      wi_pieces.append(wt)
              # ---- GEMM1: hT = relu(Wi.T @ X.T) ----
              wo_pieces = []
              HPP = H // NPIECE                                  # 256 H cols
              for p_, (i0, cnt) in enumerate(zip(WI_START, WI_SPLIT)):
                  wt = wi_pieces[p_]
                  for ir in range(0, cnt, 2):
                      # 4-way PSUM interleave over an i-chunk pair x the two
                      # n-tiles: consecutive matmuls hit 4 different banks
                      # (GEMM2's 4-deep pattern measures 238ns/matmul vs
                      # 305ns for the 2-deep version of this loop).
                      pair = [i0 + ir, i0 + ir + 1]
                      if ps2bank:
                          p2 = [pspool.tile([P, NN, NF], dt.float32,
                                            tag="ps", name=f"ps1_{i}")
                                for i in pair]
                          pts1 = [[p2[di][:, n, :] for n in range(NN)]
                                  for di in range(2)]
                      else:
                          pts1 = [
                              [
                                  pspool.tile([P, NF], dt.float32, tag="ps",
                                              name=f"ps1_{i}_{n}")
                                  for n in range(NN)
                              ]
                              for i in pair
                          ]
                      for k in range(0, KO, KSTEP):
                          for di in range(2):
                              if wide_mm:
                                  # one 1024-column matmul into the 2-bank
                                  # tile (requires x_one: rhs spans both
                                  # n-halves)
                                  nc.tensor.matmul(
                                      p2[di][:],
                                      wt[:, k:k + KSTEP,
                                         (ir + di) * P:(ir + di + 1) * P],
                                      x_whole[:, k:k + KSTEP, :],
                                      start=(k == 0),
                                      stop=(k == KO - KSTEP),
                                      perf_mode=PERF,
                                  )
                                  continue
                              for n in range(NN):
                                  nc.tensor.matmul(
                                      pts1[di][n][:],
                                      wt[:, k:k + KSTEP,
                                         (ir + di) * P:(ir + di + 1) * P],
                                      x_sbs[n][:, k:k + KSTEP, :],
                                      start=(k == 0),
                                      stop=(k == KO - KSTEP),
                                      perf_mode=PERF,
                                  )
                      for di, i in enumerate(pair):
                          hdst = h_sbs[i // ICH]
                          if ps2bank:
                              nc.vector.tensor_scalar_max(
                                  hdst[:, i % ICH, :], p2[di][:], 0.0)
                          else:
                              for n in range(NN):
                                  nc.vector.tensor_scalar_max(
                                      hdst[:, i % ICH, n * NF:(n + 1) * NF],
                                      pts1[di][n][:], 0.0
                                  )
                  # piece p_ fully consumed -> rotate the next wo piece into
                  # the freed ring slot (5 wi pieces, 4 wo pieces: skip the
                  # tiny piece 0)
                  if p_ >= 1 and do_g2:
                      g = p_ - 1
                      wot = wpool.tile([P, IC, HPP], DT, tag="w",
                                       name=f"wo{g}")
                      if do_dma:
                          nc.sync.dma_start(
                              wot[:], wo_r[:, :, g * HPP:(g + 1) * HPP])
                      wo_pieces.append(wot)

              if not do_g2:
                  return
              # ---- GEMM2: outT = Wo.T @ hT ----
              HGM = HPP // P                                     # 2 m per piece
              for g in range(NPIECE):
                  wt = wo_pieces[g]
                  if ps2bank:
                      q2 = [pspool.tile([P, NN, NF], dt.float32, tag="ps",
                                        name=f"ps2_{g}_{m}")
                            for m in range(HGM)]
                      pts = [[q2[m][:, n, :] for n in range(NN)]
                             for m in range(HGM)]
                  else:
                      pts = [
                          [
                              pspool.tile([P, NF], dt.float32, tag="ps",
                                          name=f"ps2_{g}_{m}_{n}")
                              for n in range(NN)
                          ]
                          for m in range(HGM)
                      ]
                  for k in range(0, IC, KSTEP):
                      for m in range(HGM):
                          if wide_mm:
                              nc.tensor.matmul(
                                  q2[m][:],
                                  wt[:, k:k + KSTEP, m * P:(m + 1) * P],
                                  h_sbs[k // ICH][:, k % ICH:k % ICH + KSTEP, :],
                                  start=(k == 0),
                                  stop=(k == IC - KSTEP),
                                  perf_mode=PERF,
                              )
                              continue
                          for n in range(NN):
                              nc.tensor.matmul(
                                  pts[m][n][:],
                                  wt[:, k:k + KSTEP, m * P:(m + 1) * P],
                                  h_sbs[k // ICH][
                                      :, k % ICH:k % ICH + KSTEP,
                                      n * NF:(n + 1) * NF],
                                  start=(k == 0),
                                  stop=(k == IC - KSTEP),
                                  perf_mode=PERF,
                              )
                  if not do_tail:
                      continue
                  # per-m copy + outbound DMA: each 0.5MB row leaves as soon
                  # as its copy lands, so only the last row's copy+DMA is a
                  # serial tail (~1.8us instead of ~3.5)
                  for m in range(HGM):
                      if ps2bank:
                          nc.vector.tensor_copy(
                              out_sb[:, g * HGM + m, :], q2[m][:])
                      else:
                          for n in range(NN):
                              nc.vector.tensor_copy(
                                  out_sb[:, g * HGM + m, n * NF:(n + 1) * NF],
                                  pts[m][n][:])
                      nc.sync.dma_start(
                          outT.rearrange("(ko p) n -> p ko n", p=P)[
                              :, g * HGM + m:g * HGM + m + 1, :],
                          out_sb[:, g * HGM + m:g * HGM + m + 1, :])

            if loop_repeat is not None:
                # device-side repeat loop: used only for timing (the slope
                # d(wall)/d(R) isolates per-iteration device time from the
                # ~70ms axon dispatch round-trip). hint_engines arms the
                # back-edge branch prefetch (the >256-instruction body
                # otherwise stalls ~4us on the IRAM fetch).
                with tc.For_i(0, loop_repeat, 1,
                              hint_engines=(mybir.EngineType.PE,
                                            mybir.EngineType.DVE),
                              staggered_reset=staggered):
                    _emit_body()
            else:
                for _rep in range(repeat):
                    _emit_body()
    nc.compile()
    return nc


def _get_nc(mm_dtype: str):
    if mm_dtype not in _CACHE:
        _CACHE[mm_dtype] = _build_nc(mm_dtype)
    return _CACHE[mm_dtype]


def _routing(x_flat: np.ndarray, router_w: np.ndarray, router_b: np.ndarray):
    """Replicate the reference router bit-for-bit where possible (jax CPU),
    returning top-2 values/indices [N_TOK, 2] (fp32/int)."""
    try:
        import jax
        import jax.numpy as jnp

        cpu = jax.devices("cpu")[0]
        with jax.default_device(cpu):
            xj = jnp.asarray(x_flat.reshape(B, S, H))
            logits = jnp.einsum("bsh,eh->bse", xj, jnp.asarray(router_w)) \
                + jnp.asarray(router_b)
            wflat = jax.nn.softmax(logits, axis=-1).reshape(N_TOK, E)
            topv, topi = jax.lax.top_k(wflat, TOP_K)
            return np.asarray(topv), np.asarray(topi)
    except Exception:
        # numpy fallback (float64 logits for a stable ordering)
        logits = x_flat.astype(np.float64) @ router_w.astype(np.float64).T \
            + router_b.astype(np.float64)
        m = logits.max(axis=1, keepdims=True)
        ex = np.exp(logits - m)
        wflat = (ex / ex.sum(axis=1, keepdims=True)).astype(np.float32)
        topi = np.argsort(-wflat, axis=1, kind="stable")[:, :TOP_K]
        topv = np.take_along_axis(wflat, topi, axis=1)
        return topv, topi


def _dispatch(x_flat, topv, topi):
    """Capacity-limited dispatch (exact reference order semantics).

    Returns (pos, disp_T): pos[t, e] = rank of t among selectors of e
    (token order); disp_T[e] = x of the first CAP selectors, transposed
    to [H, CAP]."""
    mask = np.zeros((N_TOK, E), dtype=bool)
    rows = np.arange(N_TOK)
    mask[rows[:, None], topi] = True
    expert_mask = mask[:, :ER]                       # [N, 7]
    pos = np.cumsum(expert_mask, axis=0, dtype=np.int32) - 1

    disp_T = np.zeros((ER, H, CAP), dtype=np.float32)
    for e in range(ER):
        idx_e = np.nonzero(expert_mask[:, e])[0][:CAP]
        disp_T[e, :, :len(idx_e)] = x_flat[idx_e].T
    return pos, disp_T


def _make_in_maps(disp_T, experts_inter, experts_out, mm_dtype=None):
    """Per-core device input maps + the output scale to undo fp8 pre-scaling."""
    import ml_dtypes

    mm_dtype = mm_dtype or MM_DTYPE
    if mm_dtype == "fp8":
        f8 = ml_dtypes.float8_e4m3
        cast_x = lambda a: np.ascontiguousarray(a.astype(f8))
        cast_wi = lambda a: np.ascontiguousarray((a * WI_SCALE).astype(f8))
        cast_wo = lambda a: np.ascontiguousarray((a * WO_SCALE).astype(f8))
        out_scale = 1.0 / (WI_SCALE * WO_SCALE)
    else:
        bf = lambda a: np.ascontiguousarray(a.astype(ml_dtypes.bfloat16))
        cast_x = cast_wi = cast_wo = bf
        out_scale = 1.0

    in_maps = []
    for c in range(N_CORES):
        e = c if c < ER else 0
        in_maps.append({
            "xT": cast_x(disp_T[e]),
            "wi": cast_wi(experts_inter[e]),
            "wo": cast_wo(experts_out[e]),
        })
    return in_maps, out_scale


def kernel(x, router_w, router_b, experts_inter, experts_out):
    from concourse.bass_utils import run_bass_kernel_spmd

    x = np.ascontiguousarray(np.asarray(x, dtype=np.float32))
    router_w = np.asarray(router_w, dtype=np.float32)
    router_b = np.asarray(router_b, dtype=np.float32)
    experts_inter = np.asarray(experts_inter, dtype=np.float32)
    experts_out = np.asarray(experts_out, dtype=np.float32)

    x_flat = x.reshape(N_TOK, H)
    topv, topi = _routing(x_flat, router_w, router_b)
    pos, disp_T = _dispatch(x_flat, topv, topi)
    rows = np.arange(N_TOK)

    mm_dtype = MM_DTYPE
    in_maps, out_scale = _make_in_maps(disp_T, experts_inter, experts_out,
                                       mm_dtype)

    nc = _get_nc(mm_dtype)
    trace = bool(int(os.environ.get("MOE_TRACE", "0")))
    res = run_bass_kernel_spmd(nc, in_maps, list(range(N_CORES)), trace=trace)
    global LAST_RESULT
    LAST_RESULT = res
    out_T = np.stack([res.results[e]["outT"] for e in range(ER)])  # [7,H,cap]

    # ---- host combine: pure gathers ----
    out_flat = np.ascontiguousarray(out_T.transpose(0, 2, 1)).reshape(
        ER * CAP, H)
    out_ext = np.vstack([out_flat, np.zeros((1, H), dtype=np.float32)])

    combined = np.zeros_like(x_flat)
    noop_w = np.zeros(N_TOK, dtype=np.float32)
    for k in range(TOP_K):
        e_k = topi[:, k]
        v_k = topv[:, k]
        is_noop = e_k == ER
        noop_w += np.where(is_noop, v_k, 0.0).astype(np.float32)
        p_k = pos[rows, np.minimum(e_k, ER - 1)]
        ok = (~is_noop) & (p_k < CAP)
        slot = np.where(ok, np.minimum(e_k, ER - 1) * CAP + p_k, ER * CAP)
        combined += out_ext[slot] * (np.where(ok, v_k, 0.0) * out_scale)[:, None]
    combined += x_flat * noop_w[:, None]

    return combined.reshape(B, S, H)



# revision 2
# speedup vs baseline: 1.1247x; 1.1247x over previous
"""MoE routing kernel for Trainium2 (8 NeuronCores, intermediate-sharded).

Problem: nn_MoDE_52140902973544 (moe_routing).
  x[4,2048,1024], router (8 experts, top-2, capacity 1024), 7 real experts
  with FFN H=1024 -> I=4096 -> H=1024 (relu), expert 7 = identity (noop).

Strategy (v2 — replaces the expert-parallel v1 where core 7 duplicated
core 0 and 1/8 of the machine was wasted):
  * Host: router forward + top-2 + capacity-limited dispatch (pure index
    math, order-based -> float-robust), gather dispatched tokens per
    expert transposed to [H, cap].
  * Device (SPMD over 8 cores): core c owns the I-slice
    [c*512, (c+1)*512) of the FFN intermediate dim for ALL 7 experts:
        h_e  = relu(x_e @ Wi_e[:, sl])          # exact (relu elementwise)
        yp_e = h_e @ Wo_e[sl, :]                # partial over I
    Every core runs the same 458,752 PE cycles (7/8 of the v1 per-core
    work) -> perfect load balance, no collectives.
  * Host: sum the 8 partial outputs (fp32), then combine via pure
    gathers (no scatter) + gate weights + noop path.

Device kernel layout (bf16 operands, fp32 PSUM):
  * Per expert e: DMA x_e [128,8,1024] (split in 2 token-halves so the
    first PSUM chain waits on ~1MB), wi-slice (2 pieces), wo-slice; all
    through double-buffered pools so expert e+1's DMA overlaps expert
    e's GEMMs.  Per-core DMA is 28MB in + 29MB out against a measured
    ~360GB/s/core (saturating HBM with all 8 cores) -> fully hidden
    behind the ~250us of PE work.
  * GEMM1 interleaves PSUM pairs (4 banks); GEMM2 likewise; drains are
    DVE ops overlapped with the next pair's matmuls; output leaves per
    2-row piece so only ~1MB of the fp32 write is a serial tail.
"""

import os
import sys

for _p in ("/opt/trn_rl_repo", "/opt/pypackages"):
    if _p not in sys.path:
        sys.path.append(_p)

import numpy as np

# ---- problem constants (hardcoded per contract) ----
B, S, H, I = 4, 2048, 1024, 4096
E = 8                 # experts incl. noop (last)
ER = E - 1            # real experts
TOP_K = 2
N_TOK = B * S         # 8192
CAP = 1024            # ceil(N_TOK / E * 1.0)
N_CORES = 8

P = 128               # partitions
KO = H // P           # 8   H chunks
ISL = I // N_CORES    # 512 I-slice per core
ICH = ISL // P        # 4   I chunks per core
NF = 512              # matmul free dim
NN = CAP // NF        # 2   cap tiles

MM_DTYPE = os.environ.get("MOE_MM_DTYPE", "bf16")

_CACHE = {}


def _build_nc(mm_dtype: str = "bf16", repeat: int = 1,
              loop_repeat: int | None = None, staggered: bool = False):
    """Single-core Bass program (SPMD across 8 cores, I-sharded).

    DRAM inputs are host-packed with the partition dim first so every
    DMA is a plain contiguous slice:
      xT [128, 7*8, 1024]  bf16   (all experts' dispatched tokens)
      wi [128, 7*8, 512]   bf16   (this core's I-slice of experts_inter)
      wo [128, 7*4, 1024]  bf16   (this core's I-slice of experts_out)
      yp [128, 7*8, 1024]  fp32   (partial outputs, summed on host)
    """
    import concourse.bacc as bacc
    import concourse.mybir as mybir
    import concourse.tile as tile

    dt = mybir.dt
    DT = dt.bfloat16

    nc = bacc.Bacc("TRN2")
    xT = nc.declare_dram_parameter("xT", [P, ER * KO, CAP], DT, isOutput=False)
    wi = nc.declare_dram_parameter("wi", [P, ER * KO, ISL], DT, isOutput=False)
    wo = nc.declare_dram_parameter("wo", [P, ER * ICH, H], DT, isOutput=False)
    yp = nc.declare_dram_parameter("yp", [P, ER * KO, CAP], dt.float32,
                                   isOutput=True)

    with tile.TileContext(nc) as tc:
        from contextlib import ExitStack

        with ExitStack() as ctx:
            xpool = ctx.enter_context(tc.tile_pool(name="x", bufs=4))
            wipool = ctx.enter_context(tc.tile_pool(name="wi", bufs=4))
            wopool = ctx.enter_context(tc.tile_pool(name="wo", bufs=2))
            hpool = ctx.enter_context(tc.tile_pool(name="h", bufs=2))
            opool = ctx.enter_context(tc.tile_pool(name="o", bufs=2))
            # shared 8-bank PSUM ring: 2-bank tiles, 4 in flight; the
            # drain of pair g overlaps the matmuls of pair g+1
            pspool = ctx.enter_context(
                tc.tile_pool(name="ps", bufs=4, space="PSUM"))

            def _emit_expert(e):
                # x in two 512-token halves: the first PSUM chain only
                # waits on a 1MB DMA, not 2MB
                xts = []
                for n in range(NN):
                    xt = xpool.tile([P, KO, NF], DT, tag="x", name=f"x{e}_{n}")
                    nc.sync.dma_start(
                        xt[:], xT[:, e * KO:(e + 1) * KO, n * NF:(n + 1) * NF])
                    xts.append(xt)
                # wi slice in two 2-chunk pieces (first GEMM1 pair waits
                # on 0.5MB)
                wits = []
                for j in range(2):
                    wit = wipool.tile([P, KO, ISL // 2], DT, tag="wi",
                                      name=f"wi{e}_{j}")
                    nc.sync.dma_start(
                        wit[:],
                        wi[:, e * KO:(e + 1) * KO,
                           j * (ISL // 2):(j + 1) * (ISL // 2)])
                    wits.append(wit)
                wot = wopool.tile([P, ICH, H], DT, tag="wo", name=f"wo{e}")
                nc.sync.dma_start(wot[:], wo[:, e * ICH:(e + 1) * ICH, :])

                ht = hpool.tile([P, ICH, CAP], DT, tag="h", name=f"h{e}")
                ot = opool.tile([P, KO, CAP], dt.float32, tag="o", name=f"o{e}")

                # ---- GEMM1: h = relu(Wi_sl.T @ X.T), I-chunk pairs ----
                for ir in range(0, ICH, 2):
                    wt = wits[ir // 2]
                    p2 = [pspool.tile([P, NN, NF], dt.float32, tag="ps",
                                      name=f"ps1_{e}_{ir + di}")
                          for di in range(2)]
                    for k in range(KO):
                        for di in range(2):
                            for n in range(NN):
                                nc.tensor.matmul(
                                    p2[di][:, n, :],
                                    wt[:, k:k + 1, di * P:(di + 1) * P],
                                    xts[n][:, k:k + 1, :],
                                    start=(k == 0),
                                    stop=(k == KO - 1),
                                )
                    for di in range(2):
                        nc.vector.tensor_scalar_max(
                            ht[:, ir + di, :], p2[di][:], 0.0)

                # ---- GEMM2: yp = Wo_sl.T @ h, H-chunk pairs ----
                for hr in range(0, KO, 2):
                    q2 = [pspool.tile([P, NN, NF], dt.float32, tag="ps",
                                      name=f"ps2_{e}_{hr + m}")
                          for m in range(2)]
                    for k in range(ICH):
                        for m in range(2):
                            for n in range(NN):
                                nc.tensor.matmul(
                                    q2[m][:, n, :],
                                    wot[:, k:k + 1,
                                        (hr + m) * P:(hr + m + 1) * P],
                                    ht[:, k:k + 1, n * NF:(n + 1) * NF],
                                    start=(k == 0),
                                    stop=(k == ICH - 1),
                                )
                    for m in range(2):
                        nc.vector.tensor_copy(ot[:, hr + m, :], q2[m][:])
                    nc.sync.dma_start(
                        yp[:, e * KO + hr:e * KO + hr + 2, :],
                        ot[:, hr:hr + 2, :])

            def _emit_body():
                for e in range(ER):
                    _emit_expert(e)

            if loop_repeat is not None:
                # device-side repeat loop for the slope timing method
                with tc.For_i(0, loop_repeat, 1,
                              hint_engines=(mybir.EngineType.PE,
                                            mybir.EngineType.DVE),
                              staggered_reset=staggered):
                    _emit_body()
            else:
                for _rep in range(repeat):
                    _emit_body()
    nc.compile()
    return nc


def _get_nc(mm_dtype: str):
    if mm_dtype not in _CACHE:
        _CACHE[mm_dtype] = _build_nc(mm_dtype)
    return _CACHE[mm_dtype]


def _routing(x_flat: np.ndarray, router_w: np.ndarray, router_b: np.ndarray):
    """Replicate the reference router bit-for-bit where possible (jax CPU),
    returning top-2 values/indices [N_TOK, 2] (fp32/int)."""
    try:
        import jax
        import jax.numpy as jnp

        cpu = jax.devices("cpu")[0]
        with jax.default_device(cpu):
            xj = jnp.asarray(x_flat.reshape(B, S, H))
            logits = jnp.einsum("bsh,eh->bse", xj, jnp.asarray(router_w)) \
                + jnp.asarray(router_b)
            wflat = jax.nn.softmax(logits, axis=-1).reshape(N_TOK, E)
            topv, topi = jax.lax.top_k(wflat, TOP_K)
            return np.asarray(topv), np.asarray(topi)
    except Exception:
        # numpy fallback (float64 logits for a stable ordering)
        logits = x_flat.astype(np.float64) @ router_w.astype(np.float64).T \
            + router_b.astype(np.float64)
        m = logits.max(axis=1, keepdims=True)
        ex = np.exp(logits - m)
        wflat = (ex / ex.sum(axis=1, keepdims=True)).astype(np.float32)
        topi = np.argsort(-wflat, axis=1, kind="stable")[:, :TOP_K]
        topv = np.take_along_axis(wflat, topi, axis=1)
        return topv, topi


def _dispatch(x_flat, topv, topi):
    """Capacity-limited dispatch (exact reference order semantics).

    Returns (pos, disp_T): pos[t, e] = rank of t among selectors of e
    (token order); disp_T[e] = x of the first CAP selectors, transposed
    to [H, CAP]."""
    mask = np.zeros((N_TOK, E), dtype=bool)
    rows = np.arange(N_TOK)
    mask[rows[:, None], topi] = True
    expert_mask = mask[:, :ER]                       # [N, 7]
    pos = np.cumsum(expert_mask, axis=0, dtype=np.int32) - 1

    disp_T = np.zeros((ER, H, CAP), dtype=np.float32)
    for e in range(ER):
        idx_e = np.nonzero(expert_mask[:, e])[0][:CAP]
        disp_T[e, :, :len(idx_e)] = x_flat[idx_e].T
    return pos, disp_T


def _make_in_maps(disp_T, experts_inter, experts_out, mm_dtype=None):
    """Per-core device input maps (I-sharded weights, replicated x)."""
    import ml_dtypes

    bf = ml_dtypes.bfloat16
    # xT: [7, H, CAP] -> [128, 7*8, 1024], same array for every core
    xT = np.ascontiguousarray(
        disp_T.reshape(ER, KO, P, CAP).transpose(2, 0, 1, 3)
        .reshape(P, ER * KO, CAP).astype(bf))
    wi_bf = experts_inter.astype(bf)     # [7, 1024, 4096]
    wo_bf = experts_out.astype(bf)       # [7, 4096, 1024]

    in_maps = []
    for c in range(N_CORES):
        sl = slice(c * ISL, (c + 1) * ISL)
        wic = np.ascontiguousarray(
            wi_bf[:, :, sl].reshape(ER, KO, P, ISL).transpose(2, 0, 1, 3)
            .reshape(P, ER * KO, ISL))
        woc = np.ascontiguousarray(
            wo_bf[:, sl, :].reshape(ER, ICH, P, H).transpose(2, 0, 1, 3)
            .reshape(P, ER * ICH, H))
        in_maps.append({"xT": xT, "wi": wic, "wo": woc})
    return in_maps, 1.0


def kernel(x, router_w, router_b, experts_inter, experts_out):
    from concourse.bass_utils import run_bass_kernel_spmd

    x = np.ascontiguousarray(np.asarray(x, dtype=np.float32))
    router_w = np.asarray(router_w, dtype=np.float32)
    router_b = np.asarray(router_b, dtype=np.float32)
    experts_inter = np.asarray(experts_inter, dtype=np.float32)
    experts_out = np.asarray(experts_out, dtype=np.float32)

    x_flat = x.reshape(N_TOK, H)
    topv, topi = _routing(x_flat, router_w, router_b)
    pos, disp_T = _dispatch(x_flat, topv, topi)
    rows = np.arange(N_TOK)

    in_maps, out_scale = _make_in_maps(disp_T, experts_inter, experts_out)

    nc = _get_nc(MM_DTYPE)
    trace = bool(int(os.environ.get("MOE_TRACE", "0")))
    res = run_bass_kernel_spmd(nc, in_maps, list(range(N_CORES)), trace=trace)
    global LAST_RESULT
    LAST_RESULT = res

    # sum the 8 partial outputs -> [7, H, CAP]
    acc = res.results[0]["yp"].astype(np.float32, copy=True)
    for c in range(1, N_CORES):
        acc += res.results[c]["yp"]
    out_T = np.ascontiguousarray(
        acc.reshape(P, ER, KO, CAP).transpose(1, 2, 0, 3).reshape(ER, H, CAP))

    # ---- host combine: pure gathers ----
    out_flat = np.ascontiguousarray(out_T.transpose(0, 2, 1)).reshape(
        ER * CAP, H)
    out_ext = np.vstack([out_flat, np.zeros((1, H), dtype=np.float32)])

    combined = np.zeros_like(x_flat)
    noop_w = np.zeros(N_TOK, dtype=np.float32)
    for k in range(TOP_K):
        e_k = topi[:, k]
        v_k = topv[:, k]
        is_noop = e_k == ER
        noop_w += np.where(is_noop, v_k, 0.0).astype(np.float32)
        p_k = pos[rows, np.minimum(e_k, ER - 1)]
        ok = (~is_noop) & (p_k < CAP)
        slot = np.where(ok, np.minimum(e_k, ER - 1) * CAP + p_k, ER * CAP)
        combined += out_ext[slot] * (np.where(ok, v_k, 0.0) * out_scale)[:, None]
    combined += x_flat * noop_w[:, None]

    return combined.reshape(B, S, H)


# revision 6
# speedup vs baseline: 1.7668x; 1.5709x over previous
"""MoE routing kernel for Trainium2 (8 NeuronCores, I-sharded, mixed prec).

Problem: nn_MoDE_52140902973544 (moe_routing).
  x[4,2048,1024], router (8 experts, top-2, capacity 1024), 7 real experts
  with FFN H=1024 -> I=4096 -> H=1024 (relu), expert 7 = identity (noop).

Strategy:
  * Host: router forward + top-2 + capacity-limited dispatch (pure index
    math, order-based -> float-robust).  Within each expert the CAP
    dispatched slots are PERMUTED by importance (gate weight x token
    norm, descending): the combine is a gather so any permutation is
    exact; it lets the device compute the low-importance tail in fp8.
  * Device (SPMD over 8 cores): core c owns I-slice [c*512,(c+1)*512)
    of the FFN intermediate dim for ALL 7 experts (perfect balance,
    458,752 bf16-equivalent PE cycles/core, no collectives):
        h_e  = relu(x_e @ Wi_e[:, sl])       # exact (relu elementwise)
        yp_e = h_e @ Wo_e[sl, :]             # partial over I, fp32 out
    Each expert's tokens split into segment A (top importance, bf16)
    and segment B (tail, fp8e4 + DoubleRow perf mode = 2x PE
    throughput).  B's quantization error lands only on slots whose
    combined contribution is small: measured end-to-end error stays
    well under the 2e-2 gate while PE work drops by NQ/2048.
  * Host: sum the 8 fp32 partials, un-scale the fp8 columns, combine
    via pure gathers + gate weights + noop path.

fp8 scales (powers of 2, exponent-only): wi8 = wi*16, wo8 = wo*64 lift
the small expert weights out of the fp8e4 subnormal range; host divides
segment-B outputs by 1024.
"""

import os
import sys

for _p in ("/opt/trn_rl_repo", "/opt/pypackages"):
    if _p not in sys.path:
        sys.path.append(_p)

import numpy as np

# ---- problem constants (hardcoded per contract) ----
B, S, H, I = 4, 2048, 1024, 4096
E = 8                 # experts incl. noop (last)
ER = E - 1            # real experts
TOP_K = 2
N_TOK = B * S         # 8192
CAP = 1024            # ceil(N_TOK / E * 1.0)
N_CORES = 8

P = 128               # partitions
KO = H // P           # 8   H chunks
ISL = I // N_CORES    # 512 I-slice per core
ICH = ISL // P        # 4   I chunks per core
NF = 512              # max matmul free dim per PSUM tile (1 bank fp32)

# tokens per expert computed in fp8 (low-importance tail); 0 = pure bf16
NQ = int(os.environ.get("MOE_NQ", "512"))
NB = CAP - NQ
WI_SCALE = 16.0
WO_SCALE = 64.0
OUT_SCALE_B = 1.0 / (WI_SCALE * WO_SCALE)

MM_DTYPE = os.environ.get("MOE_MM_DTYPE", "bf16")

_CACHE = {}


def _free_tiles(n):
    """Split a free dim of n columns into <=NF chunks: [(off, width)...]"""
    out, off = [], 0
    while off < n:
        w = min(NF, n - off)
        out.append((off, w))
        off += w
    return out


def _build_nc(mm_dtype: str = "bf16", repeat: int = 1,
              loop_repeat: int | None = None, staggered: bool = False):
    """Single-core Bass program (SPMD across 8 cores, I-sharded).

    DRAM inputs host-packed with the partition dim first so every DMA is
    a plain contiguous slice:
      xa  [128, 7*8, NB]  bf16    xb  [128, 7*8, NQ]  fp8e4
      wi  [128, 7*8, 512] bf16    wi8 [128, 7*8, 512] fp8e4 (x16)
      wo  [128, 7*4, 1024] bf16   wo8 [128, 7*4, 1024] fp8e4 (x64)
      yp  [128, 7*8, 1024] fp32 out (cols [NB:] carry the x1024 scale)
    """
    import concourse.bacc as bacc
    import concourse.mybir as mybir
    import concourse.tile as tile

    dt = mybir.dt
    F8 = dt.float8e4
    DR = mybir.MatmulPerfMode.DoubleRow

    nc = bacc.Bacc("TRN2")
    xa = nc.declare_dram_parameter("xa", [P, ER * KO, NB], dt.bfloat16,
                                   isOutput=False) if NB else None
    xb = nc.declare_dram_parameter("xb", [P, ER * KO, NQ], F8,
                                   isOutput=False) if NQ else None
    wi = nc.declare_dram_parameter("wi", [P, ER * KO, ISL], dt.bfloat16,
                                   isOutput=False) if NB else None
    wi8 = nc.declare_dram_parameter("wi8", [P, ER * KO, ISL], F8,
                                    isOutput=False) if NQ else None
    wo = nc.declare_dram_parameter("wo", [P, ER * ICH, H], dt.bfloat16,
                                   isOutput=False) if NB else None
    wo8 = nc.declare_dram_parameter("wo8", [P, ER * ICH, H], F8,
                                    isOutput=False) if NQ else None
    yp = nc.declare_dram_parameter("yp", [P, ER * KO, CAP], dt.float32,
                                   isOutput=True)

    # (tag, dtype, x dram, wi dram, wo dram, col offset, n tokens, kstep)
    segs = []
    if NB:
        segs.append(("a", dt.bfloat16, xa, wi, wo, 0, NB, 1, None))
    if NQ:
        segs.append(("b", F8, xb, wi8, wo8, NB, NQ, 2, DR))

    with tile.TileContext(nc) as tc:
        from contextlib import ExitStack

        with ExitStack() as ctx:
            xpool = ctx.enter_context(tc.tile_pool(name="x", bufs=2))
            wipool = ctx.enter_context(tc.tile_pool(name="wi", bufs=2))
            wopool = ctx.enter_context(tc.tile_pool(name="wo", bufs=2))
            hpool = ctx.enter_context(tc.tile_pool(name="h", bufs=2))
            opool = ctx.enter_context(tc.tile_pool(name="o", bufs=3))
            # 8 one-bank [P, <=512] fp32 tiles: drains of chain g overlap
            # matmuls of later chains
            pspool = ctx.enter_context(
                tc.tile_pool(name="ps", bufs=8, space="PSUM"))

            def _emit_seg(e, tag, DT, xd, wid, wod, coff, ntok, kstep, perf):
                fts = _free_tiles(ntok)
                xt = xpool.tile([P, KO, ntok], DT, tag=f"x{tag}",
                                name=f"x{tag}{e}")
                nc.sync.dma_start(xt[:], xd[:, e * KO:(e + 1) * KO, :])
                wit = wipool.tile([P, KO, ISL], DT, tag=f"wi{tag}",
                                  name=f"wi{tag}{e}")
                nc.sync.dma_start(wit[:], wid[:, e * KO:(e + 1) * KO, :])
                wot = wopool.tile([P, ICH, H], DT, tag=f"wo{tag}",
                                  name=f"wo{tag}{e}")
                nc.sync.dma_start(wot[:], wod[:, e * ICH:(e + 1) * ICH, :])

                ht = hpool.tile([P, ICH, ntok], DT, tag=f"h{tag}",
                                name=f"h{tag}{e}")

                # ---- GEMM1: h = relu(Wi_sl.T @ X.T), I-chunk pairs ----
                for ir in range(0, ICH, 2):
                    ps = [[pspool.tile([P, w], dt.float32, tag="ps",
                                       name=f"ps1{tag}_{e}_{ir + di}_{oi}")
                           for oi, (off, w) in enumerate(fts)]
                          for di in range(2)]
                    for k in range(0, KO, kstep):
                        for di in range(2):
                            for oi, (off, w) in enumerate(fts):
                                nc.tensor.matmul(
                                    ps[di][oi][:],
                                    wit[:, k:k + kstep,
                                        (ir + di) * P:(ir + di + 1) * P],
                                    xt[:, k:k + kstep, off:off + w],
                                    start=(k == 0),
                                    stop=(k == KO - kstep),
                                    perf_mode=perf,
                                )
                    for di in range(2):
                        for oi, (off, w) in enumerate(fts):
                            nc.vector.tensor_scalar_max(
                                ht[:, ir + di, off:off + w], ps[di][oi][:],
                                0.0)

                # ---- GEMM2: yp = Wo_sl.T @ h, H-chunk pairs ----
                for hr in range(0, KO, 2):
                    qs = [[pspool.tile([P, w], dt.float32, tag="ps",
                                       name=f"ps2{tag}_{e}_{hr + m}_{oi}")
                           for oi, (off, w) in enumerate(fts)]
                          for m in range(2)]
                    for k in range(0, ICH, kstep):
                        for m in range(2):
                            for oi, (off, w) in enumerate(fts):
                                nc.tensor.matmul(
                                    qs[m][oi][:],
                                    wot[:, k:k + kstep,
                                        (hr + m) * P:(hr + m + 1) * P],
                                    ht[:, k:k + kstep, off:off + w],
                                    start=(k == 0),
                                    stop=(k == ICH - kstep),
                                    perf_mode=perf,
                                )
                    ot = opool.tile([P, 2, ntok], dt.float32, tag=f"o{tag}",
                                    name=f"o{tag}{e}_{hr}")
                    for m in range(2):
                        for oi, (off, w) in enumerate(fts):
                            nc.vector.tensor_copy(
                                ot[:, m, off:off + w], qs[m][oi][:])
                    nc.sync.dma_start(
                        yp[:, e * KO + hr:e * KO + hr + 2,
                           coff:coff + ntok],
                        ot[:])

            def _emit_body():
                for e in range(ER):
                    for seg in segs:
                        _emit_seg(e, *seg)

            if loop_repeat is not None:
                # device-side repeat loop for the slope timing method
                with tc.For_i(0, loop_repeat, 1,
                              hint_engines=(mybir.EngineType.PE,
                                            mybir.EngineType.DVE),
                              staggered_reset=staggered):
                    _emit_body()
            else:
                for _rep in range(repeat):
                    _emit_body()
    nc.compile()
    return nc


def _get_nc(mm_dtype: str):
    if mm_dtype not in _CACHE:
        _CACHE[mm_dtype] = _build_nc(mm_dtype)
    return _CACHE[mm_dtype]


def _routing(x_flat: np.ndarray, router_w: np.ndarray, router_b: np.ndarray):
    """Replicate the reference router bit-for-bit where possible (jax CPU),
    returning top-2 values/indices [N_TOK, 2] (fp32/int)."""
    try:
        import jax
        import jax.numpy as jnp

        cpu = jax.devices("cpu")[0]
        with jax.default_device(cpu):
            xj = jnp.asarray(x_flat.reshape(B, S, H))
            logits = jnp.einsum("bsh,eh->bse", xj, jnp.asarray(router_w)) \
                + jnp.asarray(router_b)
            wflat = jax.nn.softmax(logits, axis=-1).reshape(N_TOK, E)
            topv, topi = jax.lax.top_k(wflat, TOP_K)
            return np.asarray(topv), np.asarray(topi)
    except Exception:
        # numpy fallback (float64 logits for a stable ordering)
        logits = x_flat.astype(np.float64) @ router_w.astype(np.float64).T \
            + router_b.astype(np.float64)
        m = logits.max(axis=1, keepdims=True)
        ex = np.exp(logits - m)
        wflat = (ex / ex.sum(axis=1, keepdims=True)).astype(np.float32)
        topi = np.argsort(-wflat, axis=1, kind="stable")[:, :TOP_K]
        topv = np.take_along_axis(wflat, topi, axis=1)
        return topv, topi


def _dispatch(x_flat, topv, topi):
    """Capacity-limited dispatch (exact reference order semantics), with
    slots permuted by importance inside each expert.

    Returns (pos, disp_T): pos[t, e] = slot column of token t for expert
    e (importance-permuted); disp_T[e] = x of the first CAP selectors in
    importance order, transposed to [H, CAP]."""
    mask = np.zeros((N_TOK, E), dtype=bool)
    rows = np.arange(N_TOK)
    mask[rows[:, None], topi] = True
    expert_mask = mask[:, :ER]                       # [N, 7]
    rank = np.cumsum(expert_mask, axis=0, dtype=np.int32) - 1
    xnorm = np.linalg.norm(x_flat, axis=1)

    pos = np.full((N_TOK, ER), CAP, dtype=np.int32)
    disp_T = np.zeros((ER, H, CAP), dtype=np.float32)
    for e in range(ER):
        idx_e = np.nonzero(expert_mask[:, e])[0][:CAP]
        w_e = np.where(topi[idx_e] == e, topv[idx_e], 0).sum(1)
        imp = w_e * xnorm[idx_e]
        perm = np.argsort(-imp, kind="stable")       # important slots first
        disp_T[e, :, :len(idx_e)] = x_flat[idx_e[perm]].T
        pos[idx_e[perm], e] = np.arange(len(idx_e), dtype=np.int32)
    return pos, disp_T


def _pack(a, dtype, nrow):
    """[ER, nrow*P, width] -> contiguous [P, ER*nrow, width] in dtype."""
    w = a.shape[-1]
    return np.ascontiguousarray(
        a.reshape(ER, nrow, P, w).transpose(2, 0, 1, 3)
        .reshape(P, ER * nrow, w).astype(dtype))


def _make_in_maps(disp_T, experts_inter, experts_out, mm_dtype=None):
    """Per-core device input maps (I-sharded weights, replicated x)."""
    import ml_dtypes

    bf = ml_dtypes.bfloat16
    f8 = ml_dtypes.float8_e4m3
    maps0 = {}
    if NB:
        maps0["xa"] = _pack(disp_T[:, :, :NB], bf, KO)
    if NQ:
        maps0["xb"] = _pack(disp_T[:, :, NB:], f8, KO)

    in_maps = []
    for c in range(N_CORES):
        sl = slice(c * ISL, (c + 1) * ISL)
        wic = np.ascontiguousarray(experts_inter[:, :, sl])
        woc = np.ascontiguousarray(experts_out[:, sl, :])
        m = dict(maps0)
        if NB:
            m["wi"] = _pack(wic, bf, KO)
            m["wo"] = _pack(woc, bf, ICH)
        if NQ:
            m["wi8"] = _pack(wic * WI_SCALE, f8, KO)
            m["wo8"] = _pack(woc * WO_SCALE, f8, ICH)
        in_maps.append(m)
    return in_maps, 1.0


def kernel(x, router_w, router_b, experts_inter, experts_out):
    from concourse.bass_utils import run_bass_kernel_spmd

    x = np.ascontiguousarray(np.asarray(x, dtype=np.float32))
    router_w = np.asarray(router_w, dtype=np.float32)
    router_b = np.asarray(router_b, dtype=np.float32)
    experts_inter = np.asarray(experts_inter, dtype=np.float32)
    experts_out = np.asarray(experts_out, dtype=np.float32)

    x_flat = x.reshape(N_TOK, H)
    topv, topi = _routing(x_flat, router_w, router_b)
    pos, disp_T = _dispatch(x_flat, topv, topi)
    rows = np.arange(N_TOK)

    in_maps, _ = _make_in_maps(disp_T, experts_inter, experts_out)

    nc = _get_nc(MM_DTYPE)
    trace = bool(int(os.environ.get("MOE_TRACE", "0")))
    res = run_bass_kernel_spmd(nc, in_maps, list(range(N_CORES)), trace=trace)
    global LAST_RESULT
    LAST_RESULT = res

    # sum the 8 partial outputs -> [7, H, CAP]; un-scale the fp8 columns
    acc = res.results[0]["yp"].astype(np.float32, copy=True)
    for c in range(1, N_CORES):
        acc += res.results[c]["yp"]
    if NQ:
        acc[:, :, NB:] *= OUT_SCALE_B
    out_T = np.ascontiguousarray(
        acc.reshape(P, ER, KO, CAP).transpose(1, 2, 0, 3).reshape(ER, H, CAP))

    # ---- host combine: pure gathers ----
    out_flat = np.ascontiguousarray(out_T.transpose(0, 2, 1)).reshape(
        ER * CAP, H)
    out_ext = np.vstack([out_flat, np.zeros((1, H), dtype=np.float32)])

    combined = np.zeros_like(x_flat)
    noop_w = np.zeros(N_TOK, dtype=np.float32)
    for k in range(TOP_K):
        e_k = topi[:, k]
        v_k = topv[:, k]
        is_noop = e_k == ER
        noop_w += np.where(is_noop, v_k, 0.0).astype(np.float32)
        p_k = pos[rows, np.minimum(e_k, ER - 1)]
        ok = (~is_noop) & (p_k < CAP)
        slot = np.where(ok, np.minimum(e_k, ER - 1) * CAP + p_k, ER * CAP)
        combined += out_ext[slot] * np.where(ok, v_k, 0.0)[:, None]
    combined += x_flat * noop_w[:, None]

    return combined.reshape(B, S, H)
